# revision 24
# baseline (speedup 1.0000x reference)
"""Trainium2 Bass kernel for nn_CrossAttention (b=2, s1=2048, s2=1024, H=16, hd=64).

Sharding: 8 cores = 2 batches x 4 head-groups (4 heads each).
Per-core device program (bf16 matmul datapath, fp32 PSUM accumulation):
  - qT proj: qT[c, s1] = q_w_g @ x_b^T            (channel-partition layout)
  - kT proj: kT[c, s2] = k_w_g @ y_b^T
  - v  proj: v[s2, c]  = y_b @ v_w_g^T            (seq-partition layout)
  - QK layernorm over head_dim (= partitions): stats via selector-matmul,
    broadcast back via tiny K=4 matmuls, applied on DVE.
  - RoPE on q in place: head-dim pre-permuted (evens then odds) on host;
    the even<->odd pair swap is done with SBUF->SBUF DMAs (engines require
    equal start partitions on all operands, DMA does not), then 3 same-base
    DVE ops with the sign baked into the sin table.
  - scoresT[s2, s1] = kT_h-slices @ qT_h          (K=64)
  - softmax without max-subtraction (|logits| <= 8 after QK-norm),
    exp on ACT with scale=hd^-0.5 fused; denominator = ones-column of v_aug.
  - oT_aug[hd+1, s1] accumulated over s2 chunks; normalized by bcast recip.
  - out-projT partial [cout, s1] = o_wt-slices @ onorm; host sums 4 partials.

The emitted BIR is post-processed to split multi-semaphore waits into
single-wait NOP chains (this walrus build allows only 1 sync-wait on
self-loading matmults and very few elsewhere).
"""

import numpy as np

B, S1, S2, CIN, H, HD = 2, 2048, 1024, 1024, 16, 64
HPC = 4                # heads per core
CPC = HPC * HD         # 256 channels per core
P = 128
KC = CIN // P          # 8 cin chunks
MC = CPC // P          # 2 channel chunks
NQ = S1 // 512         # 4 s1 slices
NK = S2 // 512         # 2 s2 slices
M2 = S2 // P           # 8 s2 chunks
SCALE = HD ** -0.5
EPS = 1e-6

_NC_CACHE = {}


def _legalize_waits(nc, mybir, limit=1):
    """Split instructions carrying >limit semaphore waits into a chain of
    single-wait NOPs on the same engine followed by the instruction."""
    n_split = 0
    for fn in nc.m.functions:
        for bb in fn.blocks:
            out = []
            for inst in bb.instructions:
                si = inst.sync_info
                waits = list(si.on_wait) if si is not None and si.on_wait else []
                if len(waits) > limit:
                    for i, w in enumerate(waits[:-limit]):
                        nop = mybir.InstNoOp(
                            name=f"{inst.name}-lw{i}", ins=[], outs=[])
                        nop.engine = inst.engine
                        nop.sync_info = mybir.SyncInfo(on_wait=[w], on_update=[])
                        try:
                            nc.register_instruction(nop, overwrite=True)
                        except Exception:
                            pass
                        out.append(nop)
                    inst.sync_info = mybir.SyncInfo(
                        on_wait=waits[-limit:], on_update=list(si.on_update))
                    n_split += 1
                out.append(inst)
            bb.instructions = out
    return n_split


def _build_nc():
    from contextlib import ExitStack

    import concourse.bass as bass
    import concourse.mybir as mybir
    import concourse.tile as tile

    f32 = mybir.dt.float32
    bf16 = mybir.dt.bfloat16
    AF = mybir.ActivationFunctionType
    OP = mybir.AluOpType

    nc = bass.Bass()

    def din(name, shape, dt=bf16):
        return nc.dram_tensor(name, list(shape), dt, kind="ExternalInput")

    xT = din("xT", (CIN, S1))
    yT = din("yT", (CIN, S2))
    qwt = din("qwt", (CIN, CPC))
    kwt = din("kwt", (CIN, CPC))
    vwt = din("vwt", (CIN, CPC))
    owt = din("owt", (CPC, CIN))
    qb = din("qb", (CPC,), f32)
    kb = din("kb", (CPC,), f32)
    vb = din("vb", (CPC,), f32)
    qnw = din("qnw", (CPC,), f32)
    qnb = din("qnb", (CPC,), f32)
    knw = din("knw", (CPC,), f32)
    knb = din("knb", (CPC,), f32)
    cosf = din("cosf", (P, S1))
    sinf = din("sinf", (P, S1))
    seld = din("sel", (CPC, HPC))
    rseld = din("rsel", (HPC, CPC))
    outT = nc.dram_tensor("outT", [CIN, S1], f32, kind="ExternalOutput")

    with tile.TileContext(nc) as tc, ExitStack() as ctx:
        consts = ctx.enter_context(tc.tile_pool(name="consts", bufs=1))
        pers = ctx.enter_context(tc.tile_pool(name="pers", bufs=1))
        xs = ctx.enter_context(tc.tile_pool(name="xs", bufs=4))
        tmp = ctx.enter_context(tc.tile_pool(name="tmp", bufs=3))
        rop = ctx.enter_context(tc.tile_pool(name="rop", bufs=3))
        expp = ctx.enter_context(tc.tile_pool(name="expp", bufs=4))
        # PSUM pools are statically allocated (8 banks total); scope them per
        # phase so proj-phase banks are freed before attention needs them.
        ctxA = ctx.enter_context(ExitStack())
        pp = ctxA.enter_context(tc.tile_pool(name="pp", bufs=4, space="PSUM"))
        pst = ctxA.enter_context(tc.tile_pool(name="pst", bufs=2, space="PSUM"))

        # ---- constants ----
        qwt_sb = consts.tile([P, KC, CPC], bf16)
        nc.sync.dma_start(out=qwt_sb, in_=qwt.rearrange("(k p) m -> p k m", p=P))
        kwt_sb = consts.tile([P, KC, CPC], bf16)
        nc.sync.dma_start(out=kwt_sb, in_=kwt.rearrange("(k p) m -> p k m", p=P))
        vwt_sb = consts.tile([P, KC, CPC], bf16)
        nc.sync.dma_start(out=vwt_sb, in_=vwt.rearrange("(k p) m -> p k m", p=P))
        owt_sb = consts.tile([P, MC, CIN], bf16)
        nc.sync.dma_start(out=owt_sb, in_=owt.rearrange("(c p) m -> p c m", p=P))
        yT_sb = consts.tile([P, KC, S2], bf16)
        nc.sync.dma_start(out=yT_sb, in_=yT.rearrange("(k p) s -> p k s", p=P))
        cosf_sb = consts.tile([P, S1], bf16)
        nc.sync.dma_start(out=cosf_sb, in_=cosf[:])
        sinf_sb = consts.tile([P, S1], bf16)
        nc.sync.dma_start(out=sinf_sb, in_=sinf[:])
        sel_sb = consts.tile([P, MC, HPC], bf16)
        nc.sync.dma_start(out=sel_sb, in_=seld.rearrange("(c p) h -> p c h", p=P))
        rsel_sb = consts.tile([HPC, CPC], bf16)
        nc.sync.dma_start(out=rsel_sb, in_=rseld[:])

        def perpart(name, d):
            t = consts.tile([P, MC], f32, name=name)
            nc.sync.dma_start(out=t, in_=d.rearrange("(c p) -> p c", p=P))
            return t

        qb_sb = perpart("qb_sb", qb)
        kb_sb = perpart("kb_sb", kb)
        qnw_sb = perpart("qnw_sb", qnw)
        qnb_sb = perpart("qnb_sb", qnb)
        knw_sb = perpart("knw_sb", knw)
        knb_sb = perpart("knb_sb", knb)
        # v bias broadcast across all partitions
        vbb_sb = consts.tile([P, CPC], f32)
        vb_ap = vb[:]
        nc.gpsimd.dma_start(
            out=vbb_sb,
            in_=bass.AP(tensor=vb_ap.tensor, offset=vb_ap.offset,
                        ap=[[0, P]] + list(vb_ap.ap)),
        )
        eps4 = consts.tile([HPC, 1], f32)
        nc.vector.memset(eps4, EPS)
        ones64 = consts.tile([1, 64], bf16)
        nc.vector.memset(ones64, 1.0)

        # ---- persistent activations ----
        qT_sb = pers.tile([P, MC, S1], bf16)
        kT_sb = pers.tile([P, MC, S2], bf16)
        v_sb = pers.tile([P, M2, HPC, HD + 1], bf16)
        onorm = pers.tile([P, MC, S1], bf16)

        # ---- q projection: qT[c*128+p, s1] ----
        # k-outer with fat 256KB x-row loads (half of S1 per pass) to keep the
        # SP HWDGE ring efficient; 4 psum accumulators live per pass.
        for half in range(2):
            hsl = slice(half * 1024, (half + 1) * 1024)
            ps = [[pp.tile([P, 512], f32, name=f"psq{c}_{half}{n}", tag="pp")
                   for n in range(2)] for c in range(MC)]
            for k in range(KC):
                xt = xs.tile([P, 1024], bf16, name=f"xt{half}_{k}", tag="xs")
                nc.sync.dma_start(out=xt, in_=xT[k * P:(k + 1) * P, hsl])
                for c in range(MC):
                    for n in range(2):
                        nc.tensor.matmul(
                            ps[c][n][:], qwt_sb[:, k, c * P:(c + 1) * P],
                            xt[:, n * 512:(n + 1) * 512],
                            start=(k == 0), stop=(k == KC - 1))
            for c in range(MC):
                for n in range(2):
                    sl = slice(half * 1024 + n * 512, half * 1024 + (n + 1) * 512)
                    nc.vector.tensor_scalar_add(qT_sb[:, c, sl], ps[c][n][:],
                                                qb_sb[:, c:c + 1])

        # ---- k projection: kT[c*128+p, s2] ----
        for n in range(NK):
            sl = slice(n * 512, (n + 1) * 512)
            ps = [pp.tile([P, 512], f32, name=f"psk{c}_{n}", tag="pp") for c in range(MC)]
            for k in range(KC):
                for c in range(MC):
                    nc.tensor.matmul(
                        ps[c][:], kwt_sb[:, k, c * P:(c + 1) * P],
                        yT_sb[:, k, sl],
                        start=(k == 0), stop=(k == KC - 1))
            for c in range(MC):
                nc.vector.tensor_scalar_add(kT_sb[:, c, sl], ps[c][:], kb_sb[:, c:c + 1])

        # ---- layernorm (stats over 64 partitions per head, via matmuls) ----
        def layernorm(src, NS, w_sb, b_sb, pfx):
            for n in range(NS):
                sl = slice(n * 512, (n + 1) * 512)
                sq = [tmp.tile([P, 512], bf16, name=f"{pfx}sq{c}_{n}", tag="sq")
                      for c in range(MC)]
                for c in range(MC):
                    nc.vector.tensor_mul(sq[c][:], src[:, c, sl], src[:, c, sl])
                pss = pst.tile([HPC, 512], f32, name=f"{pfx}pss{n}", tag="pst")
                psq = pst.tile([HPC, 512], f32, name=f"{pfx}psq{n}", tag="pst")
                for c in range(MC):
                    nc.tensor.matmul(pss[:], sel_sb[:, c, :], src[:, c, sl],
                                     start=(c == 0), stop=(c == MC - 1))
                    nc.tensor.matmul(psq[:], sel_sb[:, c, :], sq[c][:],
                                     start=(c == 0), stop=(c == MC - 1))
                mu = tmp.tile([HPC, 512], f32, name=f"{pfx}mu{n}", tag="mu", bufs=2)
                musq = tmp.tile([HPC, 512], f32, name=f"{pfx}ms{n}", tag="ms", bufs=2)
                Af = tmp.tile([HPC, 512], f32, name=f"{pfx}Af{n}", tag="Af", bufs=2)
                Ab = tmp.tile([HPC, 512], bf16, name=f"{pfx}Ab{n}", tag="Ab", bufs=2)
                Bb = tmp.tile([HPC, 512], bf16, name=f"{pfx}Bb{n}", tag="Bb", bufs=2)
                nc.vector.tensor_scalar_mul(mu[:], pss[:], 1.0 / HD)
                nc.vector.tensor_mul(musq[:], mu[:], mu[:])
                # musq <- var = psq/HD - mu^2 ; then sd = sqrt(var+eps) in place
                nc.vector.scalar_tensor_tensor(
                    out=musq[:], in0=psq[:], scalar=1.0 / HD, in1=musq[:],
                    op0=OP.mult, op1=OP.subtract)
                nc.scalar.activation(out=musq[:], in_=musq[:], func=AF.Sqrt,
                                     bias=eps4[:], scale=1.0)
                nc.vector.reciprocal(out=Af[:], in_=musq[:])      # A = rstd
                nc.vector.tensor_copy(Ab[:], Af[:])
                nc.vector.scalar_tensor_tensor(
                    out=Bb[:], in0=mu[:], scalar=-1.0, in1=Af[:],
                    op0=OP.mult, op1=OP.mult)                     # B = -mu*rstd
                # apply in place per channel chunk:
                # src = src * bcast(A)*w + (bcast(B)*w + b)
                for c in range(MC):
                    psA = pp.tile([P, 512], f32, name=f"{pfx}psA{c}_{n}", tag="pp")
                    nc.tensor.matmul(psA[:], rsel_sb[:, c * P:(c + 1) * P],
                                     Ab[:], start=True, stop=True)
                    psB = pp.tile([P, 512], f32, name=f"{pfx}psB{c}_{n}", tag="pp")
                    nc.tensor.matmul(psB[:], rsel_sb[:, c * P:(c + 1) * P],
                                     Bb[:], start=True, stop=True)
                    Ap = tmp.tile([P, 512], bf16, name=f"{pfx}Ap{c}_{n}", tag="Ap",
                                  bufs=2)
                    nc.vector.tensor_scalar_mul(Ap[:], psA[:], w_sb[:, c:c + 1])
                    Bp = tmp.tile([P, 512], bf16, name=f"{pfx}Bp{c}_{n}", tag="Bp",
                                  bufs=2)
                    nc.vector.tensor_scalar(
                        out=Bp[:], in0=psB[:], scalar1=w_sb[:, c:c + 1],
                        scalar2=b_sb[:, c:c + 1], op0=OP.mult, op1=OP.add)
                    nc.vector.tensor_mul(src[:, c, sl], src[:, c, sl], Ap[:])
                    nc.vector.tensor_add(src[:, c, sl], src[:, c, sl], Bp[:])

        layernorm(qT_sb, NQ, qnw_sb, qnb_sb, "q")

        # ---- v projection: v[s2-part, channel] + ones column ----
        # (emitted here so PE has matmul work while LN-q reciprocals run)
        for m in range(M2):
            psv = pp.tile([P, CPC], f32, name=f"psv{m}", tag="pp")
            for k in range(KC):
                nc.tensor.matmul(
                    psv[:], yT_sb[:, k, m * P:(m + 1) * P], vwt_sb[:, k, :],
                    start=(k == 0), stop=(k == KC - 1))
            nc.vector.tensor_add(
                v_sb[:, m, :, 0:HD],
                psv.rearrange("p (h d) -> p h d", h=HPC),
                vbb_sb.rearrange("p (h d) -> p h d", h=HPC))
            nc.vector.memset(v_sb[:, m, :, HD:HD + 1], 1.0)

        layernorm(kT_sb, NK, knw_sb, knb_sb, "k")

        # ---- RoPE on q, in place ----
        # Partition layout per chunk: [h_a evens | h_a odds | h_b evens | h_b odds]
        # (32 each). The even<->odd pair swap runs on DMA; then
        # q' = q*cosf + swap(q)*sinf with the sign baked into sinf.
        for c in range(MC):
            qsw = rop.tile([P, S1], bf16, name=f"qsw{c}", tag="qsw", bufs=2)
            for blk in range(4):
                d_src = (blk ^ 1) * 32          # swap evens<->odds within head
                nc.scalar.dma_start(out=qsw[blk * 32:(blk + 1) * 32, :],
                                    in_=qT_sb[d_src:d_src + 32, c, :])
            for n in range(NQ):
                sl = slice(n * 512, (n + 1) * 512)
                t = rop.tile([P, 512], bf16, name=f"rt{c}_{n}", tag="rt")
                nc.vector.tensor_mul(t[:], qsw[:, sl], sinf_sb[:, sl])
                nc.vector.tensor_mul(qT_sb[:, c, sl], qT_sb[:, c, sl],
                                     cosf_sb[:, sl])
                nc.vector.tensor_add(qT_sb[:, c, sl], qT_sb[:, c, sl], t[:])

        # ---- attention per head ----
        ctxA.close()   # free proj/LN psum banks for attention pools
        ctxB = ctx.enter_context(ExitStack())
        psc = ctxB.enter_context(tc.tile_pool(name="psc", bufs=2, space="PSUM"))
        pso = ctxB.enter_context(tc.tile_pool(name="pso", bufs=1, space="PSUM"))
        # per-head oT lands in SBUF right after its AV finishes (frees the
        # PSUM accumulator for the next head); the softmax denominators are
        # gathered into one (128, 64) tile so a single cheap full-width
        # reciprocal serves all heads/slices.
        o_sbs = [pers.tile([HD + 1, S1], f32, name=f"osb{h}") for h in range(HPC)]
        coll = pers.tile([P, HD], f32)
        for c in range(MC):
            for h2 in range(2):
                h = c * 2 + h2
                d0 = h2 * 64
                pso_t = pso.tile([HD + 1, S1], f32, name=f"pso{h}", tag="pso")
                for m in range(M2):
                    for half in range(2):
                        psc_t = psc.tile([P, 1024], f32,
                                         name=f"psc{h}_{m}_{half}", tag="psc")
                        for j in range(2):
                            n = half * 2 + j
                            nc.tensor.matmul(
                                psc_t[:, j * 512:(j + 1) * 512],
                                kT_sb[d0:d0 + 64, c, m * P:(m + 1) * P],
                                qT_sb[d0:d0 + 64, c, n * 512:(n + 1) * 512],
                                start=True, stop=True)
                        et = expp.tile([P, 1024], bf16,
                                       name=f"et{h}_{m}_{half}", tag="expp")
                        nc.scalar.activation(out=et[:], in_=psc_t[:], func=AF.Exp,
                                             scale=SCALE)
                        for j in range(2):
                            n = half * 2 + j
                            nc.tensor.matmul(
                                pso_t[:, n * 512:(n + 1) * 512],
                                v_sb[:, m, h, :],
                                et[:, j * 512:(j + 1) * 512],
                                start=(m == 0), stop=(m == M2 - 1))
                for n in range(NQ):
                    sl = slice(n * 512, (n + 1) * 512)
                    nc.vector.tensor_copy(o_sbs[h][:, sl], pso_t[:, sl])
                    r0 = h * 32 + n * 8
                    nc.gpsimd.dma_start(out=coll[r0:r0 + 8, :],
                                        in_=o_sbs[h][HD:HD + 1, sl])
        # one reciprocal for all heads' denominators, then scatter + normalize
        rcolf = pers.tile([P, HD], f32)
        rcolb = pers.tile([P, HD], bf16)
        nc.vector.reciprocal(rcolf[:], coll[:])
        nc.vector.tensor_copy(rcolb[:], rcolf[:])
        ctxB.close()   # free attention psum banks before the prb pool opens
        ctxC = ctx.enter_context(ExitStack())
        prb = ctxC.enter_context(tc.tile_pool(name="prb", bufs=3, space="PSUM"))
        for c in range(MC):
            for h2 in range(2):
                h = c * 2 + h2
                d0 = h2 * 64
                onm = None
                if h2 == 1:
                    onm = rop.tile([HD, S1], bf16, name=f"onm{h}", tag="onm",
                                   bufs=2)
                for n in range(NQ):
                    sl = slice(n * 512, (n + 1) * 512)
                    r0 = h * 32 + n * 8
                    rt1 = rop.tile([1, 512], bf16, name=f"rcp{h}_{n}", tag="rcp")
                    nc.gpsimd.dma_start(out=rt1[:], in_=rcolb[r0:r0 + 8, :])
                    prb_t = prb.tile([64, 512], f32, name=f"prb{h}_{n}", tag="prb")
                    nc.tensor.matmul(prb_t[:], ones64[:], rt1[:],
                                     start=True, stop=True)
                    if h2 == 0:
                        nc.vector.tensor_mul(onorm[0:HD, c, sl],
                                             o_sbs[h][0:HD, sl], prb_t[:])
                    else:
                        nc.vector.tensor_mul(onm[:, sl], o_sbs[h][0:HD, sl],
                                             prb_t[:])
                if h2 == 1:
                    nc.scalar.dma_start(out=onorm[HD:P, c, :], in_=onm[:])

        # ---- output projection (partial over this core's channels) ----
        ctxC.close()
        pout = ctx.enter_context(tc.tile_pool(name="pout", bufs=3, space="PSUM"))
        for mo in range(KC):
            for n in range(NQ):
                sl = slice(n * 512, (n + 1) * 512)
                po = pout.tile([P, 512], f32, name=f"po{mo}_{n}", tag="pout")
                for c in range(MC):
                    nc.tensor.matmul(po[:], owt_sb[:, c, mo * P:(mo + 1) * P],
                                     onorm[:, c, sl],
                                     start=(c == 0), stop=(c == MC - 1))
                ost = xs.tile([P, 512], f32, name=f"ost{mo}_{n}", tag="ost")
                nc.vector.tensor_copy(ost[:], po[:])
                nc.scalar.dma_start(out=outT[mo * P:(mo + 1) * P, sl], in_=ost[:])

    _legalize_waits(nc, mybir, limit=1)
    return nc


def get_nc():
    if "nc" not in _NC_CACHE:
        _NC_CACHE["nc"] = _build_nc()
    return _NC_CACHE["nc"]


def make_in_maps(x, y, q_w, q_b, kv_w, kv_b, qn_w, qn_b, kn_w, kn_b, out_w, out_b):
    import ml_dtypes
    bf = ml_dtypes.bfloat16
    perm = np.concatenate([np.arange(0, HD, 2), np.arange(1, HD, 2)])
    inv_freq = (1.0 / (10000.0 ** (np.arange(0, HD, 2, dtype=np.float32)
                                   / np.float32(HD)))).astype(np.float32)
    ang = np.arange(S1, dtype=np.float32)[None, :] * inv_freq[:, None]
    cos = np.cos(ang).astype(np.float32)           # (32, S1)
    sin = np.sin(ang).astype(np.float32)
    cosf = np.tile(cos, (4, 1)).astype(bf)
    sinf = np.concatenate([-sin, sin, -sin, sin]).astype(bf)
    sel = np.zeros((CPC, HPC), np.float32)
    for h in range(HPC):
        sel[h * HD:(h + 1) * HD, h] = 1.0
    rsel = np.ascontiguousarray(sel.T).astype(bf)
    sel = sel.astype(bf)

    in_maps = []
    for core in range(8):
        b, g = divmod(core, 4)
        heads = [HPC * g + i for i in range(HPC)]
        qrows = np.concatenate([h * HD + perm for h in heads])
        vrows = np.concatenate([CIN + h * HD + np.arange(HD) for h in heads])
        ocols = np.concatenate([h * HD + np.arange(HD) for h in heads])
        in_maps.append({
            "xT": np.ascontiguousarray(x[b].T).astype(bf),
            "yT": np.ascontiguousarray(y[b].T).astype(bf),
            "qwt": np.ascontiguousarray(q_w[qrows].T).astype(bf),
            "kwt": np.ascontiguousarray(kv_w[qrows].T).astype(bf),
            "vwt": np.ascontiguousarray(kv_w[vrows].T).astype(bf),
            "owt": np.ascontiguousarray(out_w[:, ocols].T).astype(bf),
            "qb": np.ascontiguousarray(q_b[qrows]),
            "kb": np.ascontiguousarray(kv_b[qrows]),
            "vb": np.ascontiguousarray(kv_b[vrows]),
            "qnw": np.ascontiguousarray(np.tile(qn_w[perm], HPC)),
            "qnb": np.ascontiguousarray(np.tile(qn_b[perm], HPC)),
            "knw": np.ascontiguousarray(np.tile(kn_w[perm], HPC)),
            "knb": np.ascontiguousarray(np.tile(kn_b[perm], HPC)),
            "cosf": cosf, "sinf": sinf, "sel": sel, "rsel": rsel,
        })
    return in_maps


def assemble(parts, out_b):
    result = np.empty((B, S1, CIN), np.float32)
    for b in range(B):
        acc = parts[b * 4].astype(np.float32)
        for g in range(1, 4):
            acc = acc + parts[b * 4 + g]
        result[b] = acc.T + out_b[None, :].astype(np.float32)
    return result


def kernel(**inputs):
    args = {k: np.asarray(inputs[k], np.float32) for k in
            ("x", "y", "q_w", "q_b", "kv_w", "kv_b", "qn_w", "qn_b",
             "kn_w", "kn_b", "out_w", "out_b")}
    in_maps = make_in_maps(
        args["x"], args["y"], args["q_w"], args["q_b"], args["kv_w"],
        args["kv_b"], args["qn_w"], args["qn_b"], args["kn_w"], args["kn_b"],
        args["out_w"], args["out_b"])
    from concourse.bass_utils import run_bass_kernel_spmd
    nc = get_nc()
    res = run_bass_kernel_spmd(nc, in_maps, core_ids=list(range(8)))
    parts = [r["outT"] for r in res.results]
    return assemble(parts, args["out_b"])
